# revision 3
# baseline (speedup 1.0000x reference)
"""CP-decomposed embedding lookup kernel for Trainium2 (8 NeuronCores).

Math (matches the CPEmbedding reference):
    A = khatri_rao(U0, U1, U2)            # [500000, 32]
    B = khatri_rao(V0, V1)                # [128, 32]
    out = (A @ B.T)[x]                    # [1024, 200, 128]

We never materialize A. Per lookup x = a*5000 + b*50 + c = j*50 + c:
    wT[r]  = U01T[r, j] * U2T[r, c]       where U01T[r, a*100+b] = U0[a,r]*U1[b,r]
    out[x] = wT.T @ B.T

Key change vs the SWDGE baseline: the gathers run as gpsimd `ap_gather`
(SBUF->SBUF free-dim gather in Q7 ucode, no per-row DMA descriptors).  The
U01T table is built once in SBUF ([128 part, 10000] f32, 40KB/partition,
rank r = p%32 replicated at the 4 32-partition bands), so each band serves
one quarter of the core's lookups and the gather lands already transposed
([rank, lookup]) -- no PE transposes, and the final matmuls read it as lhsT
directly.

Sharding: CP factors replicated; the 204800 lookups are sharded evenly
across the 8 cores (data parallel), each core computing a contiguous
[25600, 128] output slice; host concatenates.

Per-core pipeline, 5 chunks of 1280 lookups/quarter (5120 total):
  ap_gather j-rows + c-rows -> DVE mult -> WT [128, 1280]
  -> per 128-lookup tile: fp32 matmul lhsT=WT[band, tile], rhs=B^T[band]
     (tile_position row bands, own PSUM bank each)
  -> ACT/DVE copies PSUM->SBUF staging -> one 2.6MB HWDGE DMA per chunk.
"""

import numpy as np

import concourse.bacc as bacc
import concourse.mybir as mybir
import concourse.tile as tile
from concourse import bass_utils

# Problem constants (hardcoded per the harness contract).
VOC = (100, 100, 50)  # a, b, c
EMB = (8, 16)  # d, e'
RANK = 32
E = EMB[0] * EMB[1]  # 128
N_CORES = 8
X_SHAPE = (1024, 200)
N_TOTAL = X_SHAPE[0] * X_SHAPE[1]  # 204800
P = 128

N_CORE = N_TOTAL // N_CORES  # 25600 lookups per core
NQ = 4  # lookup quarters, one per 32-partition rank band
N_QUARTER = N_CORE // NQ  # 6400
U01_COLS = VOC[0] * VOC[1]  # 10000

F32 = mybir.dt.float32
I16 = mybir.dt.int16


class Cfg:
    def __init__(self, chunk):
        assert N_QUARTER % chunk == 0
        assert chunk % P == 0 and chunk % 16 == 0
        self.chunk = chunk  # lookups per quarter per chunk
        self.n_chunks = N_QUARTER // chunk
        self.tpc = chunk // P  # tiles per quarter per chunk
        self.icols = chunk // 16  # idx cols per chunk


FULL_CFG = Cfg(1280)


def build_program(cfg: Cfg):
    nc = bacc.Bacc("TRN2", target_bir_lowering=False, debug=False)

    # ---- DRAM I/O ----
    jidx_d = nc.dram_tensor("jidx", [P, N_QUARTER // 16], I16, kind="ExternalInput")
    cidx_d = nc.dram_tensor("cidx", [P, N_QUARTER // 16], I16, kind="ExternalInput")
    u0t4_d = nc.dram_tensor("u0t4", [P, VOC[0]], F32, kind="ExternalInput")
    u1t4_d = nc.dram_tensor("u1t4", [P, VOC[1]], F32, kind="ExternalInput")
    u2t4_d = nc.dram_tensor("u2t4", [P, VOC[2]], F32, kind="ExternalInput")
    v0t4_d = nc.dram_tensor("v0t4", [P, EMB[0]], F32, kind="ExternalInput")
    v1t4_d = nc.dram_tensor("v1t4", [P, EMB[1]], F32, kind="ExternalInput")
    out_d = nc.dram_tensor("out", [N_CORE, E], F32, kind="ExternalOutput")

    with tile.TileContext(nc) as tc:
        const = tc.alloc_tile_pool(name="const", bufs=1)

        # ---------- one-time setup ----------
        u0t4 = const.tile([P, VOC[0]], F32)
        u1t4 = const.tile([P, VOC[1]], F32)
        u2t4 = const.tile([P, VOC[2]], F32)
        jidx = const.tile([P, N_QUARTER // 16], I16)
        cidx = const.tile([P, N_QUARTER // 16], I16)
        v0t4 = const.tile([P, EMB[0]], F32)
        v1t4 = const.tile([P, EMB[1]], F32)
        nc.sync.dma_start(u0t4[:], u0t4_d.ap())
        nc.sync.dma_start(u1t4[:], u1t4_d.ap())
        nc.scalar.dma_start(u2t4[:], u2t4_d.ap())
        nc.scalar.dma_start(jidx[:], jidx_d.ap())
        nc.scalar.dma_start(cidx[:], cidx_d.ap())
        nc.scalar.dma_start(v0t4[:], v0t4_d.ap())
        nc.scalar.dma_start(v1t4[:], v1t4_d.ap())

        # U01T[32q + r, a*100 + b] = U0[a, r] * U1[b, r]; the j-gather table.
        # Built in 4 a-range slabs so the DVE work pipelines with the input
        # DMAs and the b-side broadcast.
        u01t = const.tile([P, U01_COLS], F32)
        NB = 4
        AH = VOC[0] // NB
        for h in range(NB):
            asl = slice(h * AH, (h + 1) * AH)
            nc.vector.tensor_tensor(
                out=u01t[:].rearrange("p (a b) -> p a b", b=VOC[1])[:, asl, :],
                in0=u0t4[:][:, asl, None].to_broadcast([P, AH, VOC[1]]),
                in1=u1t4[:][:, None, :].to_broadcast([P, AH, VOC[1]]),
                op=mybir.AluOpType.mult,
            )

        # B^T replicated at the 4 partition bands:
        # bt[32q + r, d*16 + e'] = V0[d, r] * V1[e', r]
        bt = const.tile([P, E], F32)
        nc.vector.tensor_tensor(
            out=bt[:].rearrange("p (d e) -> p d e", e=EMB[1]),
            in0=v0t4[:][:, :, None].to_broadcast([P, EMB[0], EMB[1]]),
            in1=v1t4[:][:, None, :].to_broadcast([P, EMB[0], EMB[1]]),
            op=mybir.AluOpType.mult,
        )

        # ---------- main pipeline ----------
        g1p = tc.alloc_tile_pool(name="g1", bufs=2)
        g2p = tc.alloc_tile_pool(name="g2", bufs=2)
        wp = tc.alloc_tile_pool(name="w", bufs=2)
        # fp32 matmuls sharing a PSUM bank crash the exec unit; one bank per
        # in-flight matmul.
        opp = tc.alloc_tile_pool(name="op", bufs=6, space="PSUM")
        osp = tc.alloc_tile_pool(name="os", bufs=2)

        for ch in range(cfg.n_chunks):
            c0 = ch * cfg.icols
            g1 = g1p.tile([P, cfg.chunk], F32, tag="g1")
            g2 = g2p.tile([P, cfg.chunk], F32, tag="g2")
            # Each 16-partition group gathers its quarter's indices from the
            # SBUF-resident tables; no DMA descriptors involved.
            nc.gpsimd.ap_gather(
                g2[:], u2t4[:], cidx[:][:, c0 : c0 + cfg.icols],
                channels=P, num_elems=VOC[2], d=1, num_idxs=cfg.chunk,
            )
            nc.gpsimd.ap_gather(
                g1[:], u01t[:], jidx[:][:, c0 : c0 + cfg.icols],
                channels=P, num_elems=U01_COLS, d=1, num_idxs=cfg.chunk,
            )
            w = wp.tile([P, cfg.chunk], F32, tag="w")
            nc.vector.tensor_tensor(
                out=w[:], in0=g1[:], in1=g2[:], op=mybir.AluOpType.mult
            )
            out_sb = osp.tile([P, NQ * cfg.tpc * E], F32, tag="os")
            for t in range(cfg.tpc):
                for q in range(NQ):
                    out_ps = opp.tile([P, E], F32, tag="ops")
                    nc.tensor.matmul(
                        out=out_ps[:],
                        lhsT=w[:][q * RANK : (q + 1) * RANK, t * P : (t + 1) * P],
                        rhs=bt[:][q * RANK : (q + 1) * RANK, :],
                        start=True,
                        stop=True,
                        tile_position=(q * RANK, 0),
                    )
                    dst = out_sb[:][:, (q * cfg.tpc + t) * E : (q * cfg.tpc + t + 1) * E]
                    if (t * NQ + q) % 2 == 0:
                        nc.scalar.copy(dst, out_ps[:])
                    else:
                        nc.vector.tensor_copy(dst, out_ps[:])
            # Rows q*6400 + tg*128 + p; one 3-dim DMA per quarter (the AP
            # balancer can't merge the 4-dim form).
            for q in range(NQ):
                row0 = q * N_QUARTER + ch * cfg.tpc * P
                nc.sync.dma_start(
                    out_d.ap()[row0 : row0 + cfg.tpc * P, :].rearrange(
                        "(t p) e -> p t e", p=P
                    ),
                    out_sb[:][:, q * cfg.tpc * E : (q + 1) * cfg.tpc * E].rearrange(
                        "p (t e) -> p t e", e=E
                    ),
                )

        for pool in (osp, opp, wp, g2p, g1p, const):
            pool.release()

    nc.compile()
    return nc


def wrap_idx_quarters(v: np.ndarray) -> np.ndarray:
    """[25600] int16 -> [128, 1600]: quarter q wrapped [i%16, i//16] into
    partitions 32q..32q+15 and replicated at 32q+16..32q+31 (the two
    16-partition gpsimd groups of rank band q)."""
    rows = []
    vq = v.reshape(NQ, N_QUARTER)
    for q in range(NQ):
        w = vq[q].reshape(-1, 16).T  # [16, 400]
        rows.append(np.tile(w, (2, 1)))  # [32, 400]
    return np.ascontiguousarray(np.concatenate(rows, axis=0))


_CACHE: dict = {}


def _get_program(cfg: Cfg):
    key = ("apg", cfg.chunk)
    if key not in _CACHE:
        _CACHE[key] = build_program(cfg)
    return _CACHE[key]


def make_in_maps(x, U0, U1, U2, V0, V1):
    xf = np.asarray(x).reshape(-1).astype(np.int64)
    j = (xf // VOC[2]).astype(np.int16)  # [0, 10000)
    c = (xf % VOC[2]).astype(np.int16)  # [0, 50)

    u0t4 = np.ascontiguousarray(np.tile(np.asarray(U0, np.float32).T, (NQ, 1)))
    u1t4 = np.ascontiguousarray(np.tile(np.asarray(U1, np.float32).T, (NQ, 1)))
    u2t4 = np.ascontiguousarray(np.tile(np.asarray(U2, np.float32).T, (NQ, 1)))
    v0t4 = np.ascontiguousarray(np.tile(np.asarray(V0, np.float32).T, (NQ, 1)))
    v1t4 = np.ascontiguousarray(np.tile(np.asarray(V1, np.float32).T, (NQ, 1)))

    in_maps = []
    for k in range(N_CORES):
        sl = slice(k * N_CORE, (k + 1) * N_CORE)
        in_maps.append(
            {
                "jidx": wrap_idx_quarters(j[sl]),
                "cidx": wrap_idx_quarters(c[sl]),
                "u0t4": u0t4,
                "u1t4": u1t4,
                "u2t4": u2t4,
                "v0t4": v0t4,
                "v1t4": v1t4,
            }
        )
    return in_maps


def kernel(x, U0, U1, U2, V0, V1, _trace=False):
    cfg = FULL_CFG
    nc = _get_program(cfg)
    in_maps = make_in_maps(x, U0, U1, U2, V0, V1)
    res = bass_utils.run_bass_kernel_spmd(
        nc, in_maps, core_ids=list(range(N_CORES)), trace=_trace
    )
    out = np.concatenate([res.results[k]["out"] for k in range(N_CORES)], axis=0)
    out = out.reshape(*np.asarray(x).shape, E).astype(np.float32)
    if _trace:
        kernel._last_result = res
    return out


# revision 11
# speedup vs baseline: 2.0124x; 2.0124x over previous
"""CP-decomposed embedding lookup kernel for Trainium2 (8 NeuronCores).

Math (matches the CPEmbedding reference):
    A = khatri_rao(U0, U1, U2)            # [500000, 32]
    B = khatri_rao(V0, V1)                # [128, 32]
    out = (A @ B.T)[x]                    # [1024, 200, 128]

Lookup x = a*5000 + b*50 + c.  All per-lookup row selection runs on the
TENSOR engine as one-hot matmuls (every GPSIMD gather path on TRN2 is
latency-capped at ~100 cycles per non-pipelined SBUF read command, which
caps any Q7 gather at ~27ns/index):

    rep_v  = ones.T @ v_row          # K=1 matmul: index row -> 128 partitions
    oh_v   = (rep_v == iota_p)       # DVE compare against partition index
    WvT    = Uv.T(bf16) @ oh_v       # [32, 512] = exactly Uv[v_p, :] per column
    WT     = W0T * W1T * W2T         # DVE, psum -> sbuf bf16
    out    = WT.T @ B^T              # per 128-lookup tile, bf16, f32 psum

Blocks of 512 lookups; 4 blocks pack the 4 32-partition rank bands of a
[128, 512] superblock so DVE/PE run full-width.  Output staged in SBUF and
written with one ~1MB HWDGE DMA per superblock.  Lookups are sharded
contiguously across the 8 cores (data parallel); factors are replicated.
"""

import ml_dtypes
import numpy as np

import concourse.bacc as bacc
import concourse.mybir as mybir
import concourse.tile as tile
from concourse import bass_utils

# Problem constants (hardcoded per the harness contract).
VOC = (100, 100, 50)  # a, b, c
EMB = (8, 16)  # d, e'
RANK = 32
E = EMB[0] * EMB[1]  # 128
N_CORES = 8
X_SHAPE = (1024, 200)
N_TOTAL = X_SHAPE[0] * X_SHAPE[1]  # 204800
P = 128

N_CORE = N_TOTAL // N_CORES  # 25600 lookups per core
BLK = 512  # lookups per block (PSUM bank N limit)
NB_FULL = 4  # blocks per full superblock (one per rank band)
N_BLOCKS = N_CORE // BLK  # 50
# superblocks: 12 x 4 blocks + 1 x 2 blocks
SBS = [(m * NB_FULL, NB_FULL) for m in range(N_BLOCKS // NB_FULL)]
if N_BLOCKS % NB_FULL:
    SBS.append((N_BLOCKS - N_BLOCKS % NB_FULL, N_BLOCKS % NB_FULL))

F32 = mybir.dt.float32
BF16 = mybir.dt.bfloat16
I32 = mybir.dt.int32


def build_program():
    nc = bacc.Bacc("TRN2", target_bir_lowering=False, debug=False)

    arow_d = nc.dram_tensor("arow", [1, N_CORE], BF16, kind="ExternalInput")
    brow_d = nc.dram_tensor("brow", [1, N_CORE], BF16, kind="ExternalInput")
    crow_d = nc.dram_tensor("crow", [1, N_CORE], BF16, kind="ExternalInput")
    u0b_d = nc.dram_tensor("u0b", [VOC[0], RANK], BF16, kind="ExternalInput")
    u1b_d = nc.dram_tensor("u1b", [VOC[1], RANK], BF16, kind="ExternalInput")
    u2b_d = nc.dram_tensor("u2b", [VOC[2], RANK], BF16, kind="ExternalInput")
    ones_d = nc.dram_tensor("ones", [1, P], BF16, kind="ExternalInput")
    v0t4_d = nc.dram_tensor("v0t4", [P, EMB[0]], F32, kind="ExternalInput")
    v1t4_d = nc.dram_tensor("v1t4", [P, EMB[1]], F32, kind="ExternalInput")
    out_d = nc.dram_tensor("out", [N_CORE, E], F32, kind="ExternalOutput")

    with tile.TileContext(nc) as tc:
        const = tc.alloc_tile_pool(name="const", bufs=1)

        arow = const.tile([1, N_CORE], BF16)
        brow = const.tile([1, N_CORE], BF16)
        crow = const.tile([1, N_CORE], BF16)
        u0b = const.tile([VOC[0], RANK], BF16)
        u1b = const.tile([VOC[1], RANK], BF16)
        u2b = const.tile([VOC[2], RANK], BF16)
        ones = const.tile([1, P], BF16)
        v0t4 = const.tile([P, EMB[0]], F32)
        v1t4 = const.tile([P, EMB[1]], F32)
        nc.sync.dma_start(arow[:], arow_d.ap())
        nc.sync.dma_start(brow[:], brow_d.ap())
        nc.scalar.dma_start(crow[:], crow_d.ap())
        nc.scalar.dma_start(u0b[:], u0b_d.ap())
        nc.scalar.dma_start(u1b[:], u1b_d.ap())
        nc.scalar.dma_start(u2b[:], u2b_d.ap())
        nc.scalar.dma_start(ones[:], ones_d.ap())
        nc.scalar.dma_start(v0t4[:], v0t4_d.ap())
        nc.scalar.dma_start(v1t4[:], v1t4_d.ap())

        # [128, 512] tile of the partition index, for the one-hot compares
        iota32 = const.tile([P, BLK], I32)
        nc.gpsimd.iota(iota32[:], pattern=[[0, BLK]], base=0, channel_multiplier=1)
        iotaf = const.tile([P, BLK], F32)
        nc.vector.tensor_copy(iotaf[:], iota32[:])

        # B^T replicated at the 4 rank bands: bt[32q+r, d*16+e'] = V0[d,r]V1[e',r]
        bt = const.tile([P, E], BF16)
        nc.vector.tensor_tensor(
            out=bt[:].rearrange("p (d e) -> p d e", e=EMB[1]),
            in0=v0t4[:][:, :, None].to_broadcast([P, EMB[0], EMB[1]]),
            in1=v1t4[:][:, None, :].to_broadcast([P, EMB[0], EMB[1]]),
            op=mybir.AluOpType.mult,
        )

        repp = tc.alloc_tile_pool(name="rep", bufs=3, space="PSUM")
        ohp = tc.alloc_tile_pool(name="oh", bufs=2)
        fpp = tc.alloc_tile_pool(name="fp", bufs=1, space="PSUM")
        t1p = tc.alloc_tile_pool(name="t1", bufs=2)
        wtp = tc.alloc_tile_pool(name="wt", bufs=2)
        opp = tc.alloc_tile_pool(name="op", bufs=2, space="PSUM")
        osp = tc.alloc_tile_pool(name="os", bufs=2)

        FACTORS = ((arow, u0b, VOC[0]), (brow, u1b, VOC[1]), (crow, u2b, VOC[2]))

        for blk0, nb in SBS:
            # ---- build WT [32*nb, nb*512 lookups] ----
            w0ps = fpp.tile([P, BLK], F32, tag="w0")
            w1ps = fpp.tile([P, BLK], F32, tag="w1")
            w2ps = fpp.tile([P, BLK], F32, tag="w2")
            wps = [w0ps, w1ps, w2ps]
            for b in range(nb):
                cols = slice((blk0 + b) * BLK, (blk0 + b + 1) * BLK)
                for f, (row, ub, voc) in enumerate(FACTORS):
                    rep = repp.tile([P, BLK], F32, tag="rep")
                    nc.tensor.matmul(
                        out=rep[:],
                        lhsT=ones[:],
                        rhs=row[:][:, cols],
                        start=True,
                        stop=True,
                    )
                    oh = ohp.tile([P, BLK], BF16, tag=f"oh{f}")
                    nc.vector.tensor_tensor(
                        out=oh[:][0:voc, :],
                        in0=rep[:][0:voc, :],
                        in1=iotaf[:][0:voc, :],
                        op=mybir.AluOpType.is_equal,
                    )
                    nc.tensor.matmul(
                        out=wps[f][:][b * RANK : (b + 1) * RANK, :],
                        lhsT=ub[:],
                        rhs=oh[:][0:voc, :],
                        start=True,
                        stop=True,
                        tile_position=(0, b * RANK),
                    )
            # DVE tensor_tensor can read at most one PSUM operand -> stage w1
            # through SBUF on the ACT engine.
            pr = nb * RANK
            w1sb = t1p.tile([P, BLK], BF16, tag="w1sb")
            nc.scalar.copy(w1sb[:][0:pr, :], wps[1][:][0:pr, :])
            t1 = t1p.tile([P, BLK], BF16, tag="t1")
            nc.vector.tensor_tensor(
                out=t1[:][0:pr, :],
                in0=wps[0][:][0:pr, :],
                in1=w1sb[:][0:pr, :],
                op=mybir.AluOpType.mult,
            )
            wt = wtp.tile([P, BLK], BF16, tag="wtile")
            nc.vector.tensor_tensor(
                out=wt[:][0:pr, :],
                in0=wps[2][:][0:pr, :],
                in1=t1[:][0:pr, :],
                op=mybir.AluOpType.mult,
            )
            # ---- final matmuls + staging + one DMA ----
            out_sb = osp.tile([P, nb * 4 * E], F32, tag="os")
            for b in range(nb):
                out_ps = opp.tile([P, 4 * E], F32, tag="ops")
                for t in range(4):
                    nc.tensor.matmul(
                        out=out_ps[:][:, t * E : (t + 1) * E],
                        lhsT=wt[:][b * RANK : (b + 1) * RANK, t * P : (t + 1) * P],
                        rhs=bt[:][b * RANK : (b + 1) * RANK, :],
                        start=True,
                        stop=True,
                        tile_position=(b * RANK, 0),
                    )
                nc.scalar.copy(out_sb[:][:, b * 4 * E : (b + 1) * 4 * E], out_ps[:])
            row0 = blk0 * BLK
            nc.sync.dma_start(
                out_d.ap()[row0 : row0 + nb * BLK, :].rearrange(
                    "(s p) e -> p s e", p=P
                ),
                out_sb[:].rearrange("p (s e) -> p s e", e=E),
            )

        for pool in (osp, opp, wtp, t1p, fpp, ohp, repp, const):
            pool.release()

    nc.compile()
    return nc


_CACHE: dict = {}


def _get_program():
    if "onehot" not in _CACHE:
        _CACHE["onehot"] = build_program()
    return _CACHE["onehot"]


def make_in_maps(x, U0, U1, U2, V0, V1):
    bf = ml_dtypes.bfloat16
    xf = np.asarray(x).reshape(-1).astype(np.int64)
    a = (xf // (VOC[1] * VOC[2])).astype(np.float32)
    b = ((xf // VOC[2]) % VOC[1]).astype(np.float32)
    c = (xf % VOC[2]).astype(np.float32)

    u0b = np.asarray(U0, np.float32).astype(bf)
    u1b = np.asarray(U1, np.float32).astype(bf)
    u2b = np.asarray(U2, np.float32).astype(bf)
    ones = np.ones((1, P), dtype=bf)
    v0t4 = np.ascontiguousarray(np.tile(np.asarray(V0, np.float32).T, (4, 1)))
    v1t4 = np.ascontiguousarray(np.tile(np.asarray(V1, np.float32).T, (4, 1)))

    in_maps = []
    for k in range(N_CORES):
        sl = slice(k * N_CORE, (k + 1) * N_CORE)
        in_maps.append(
            {
                "arow": a[sl].reshape(1, N_CORE).astype(bf),
                "brow": b[sl].reshape(1, N_CORE).astype(bf),
                "crow": c[sl].reshape(1, N_CORE).astype(bf),
                "u0b": u0b,
                "u1b": u1b,
                "u2b": u2b,
                "ones": ones,
                "v0t4": v0t4,
                "v1t4": v1t4,
            }
        )
    return in_maps


def kernel(x, U0, U1, U2, V0, V1, _trace=False):
    nc = _get_program()
    in_maps = make_in_maps(x, U0, U1, U2, V0, V1)
    res = bass_utils.run_bass_kernel_spmd(
        nc, in_maps, core_ids=list(range(N_CORES)), trace=_trace
    )
    out = np.concatenate([res.results[k]["out"] for k in range(N_CORES)], axis=0)
    out = out.reshape(*np.asarray(x).shape, E).astype(np.float32)
    if _trace:
        kernel._last_result = res
    return out
